# revision 25
# baseline (speedup 1.0000x reference)
"""Trainium2 Bass kernel for a 3-layer difflogic network (nn_Net_48610439856713).

Math: each layer o computes softmax(w[o])·ops16(a, b) with a = h[:, ia[o]],
b = h[:, ib[o]].  The 16 relaxed logic gates are affine in {1, a, b, ab},
so a layer is h' = C0 + C1·a + C2·b + C3·a·b with per-neuron coefficients
derived on-device from softmax(w).

Design (v3):
 - 8 cores = 2 batch groups x 4 neuron shards.  Activations and combine
   arithmetic in fp8(e4m3): the final GroupSum averages ~5333 neurons, so
   quantization noise cancels (validated ~7e-3 final rel err vs 2e-2 budget).
 - a-stream gathers run on the Tensor engine as one-hot matmuls: each layer's
   outputs are assigned to the core that owns their a-source shard and sorted
   by source slot, so each 128-output tile reads only NB source j-columns
   (3 for layer 2, 4 for layer 3).  One-hots are host-packed fp8 {0,1} data.
   Layer 1 (source x, 193 rows = 2 K-blocks) runs both streams on PE.
 - b-stream gathers (random remote rows) use SWDGE dma_gather from the
   AllGathered fp8 buffer.  Descriptor generation is hoisted off the critical
   path with prepare_only + trigger_dma; each layer's gather is split into
   4 sub-gathers on 4 SWDGE queues (the drain is latency-bound on random
   256B HBM reads, and parallel queue rings multiply outstanding requests).
 - Combine: u = C1*a + C0 runs per-j on the Scalar engine (per-partition
   scale/bias APs); the remaining 5 passes are full-chunk fp8 DVE ops
   (t = ab, t *= C3, v = b*C2, t += v, h = t + u), with the C3/C2 factors
   as stride-0 broadcast APs.
 - GroupSum = 0/1-mask matmuls on PE into PSUM (interleaved with the layer-3
   combine), then a tiny AllGather and partition-0 adds.

Host-side work is integer/layout only: slot permutations, one-hot placement
(bytes 0x00/0x38), index relabeling, int16 wrapping, weight-row packing.
All float arithmetic (softmax, combine, sums) runs on device.
"""

import os
import numpy as np
import ml_dtypes

FP8 = ml_dtypes.float8_e4m3fn

P = 128
B = 512
BG = 2                  # batch groups
SH = 4                  # neuron shards
BC = B // BG            # 256 batch per core
IN = 193
NGROUP = 3
TAU = 100.0
N_CORES = 8
NB2 = 3                 # source j-blocks per output tile, layer 2
NB3 = 4                 # layer 3

NJ1, R1 = 32, 4000      # layer1: 4000 real rows/shard, 4096 slots
NS1 = NJ1 * P
NJ2 = 34
NS2 = NJ2 * P           # 4352 slots (binomial 4000+-55, 6.4 sigma margin)
NJ3 = 34
NS3 = NJ3 * P
NCH = 2                 # AllGather chunks per layer
JCH1, JCH2, JCH3 = NJ1 // NCH, NJ2 // NCH, NJ3 // NCH
L1N = 16000
L3N = 15999
SPG = L3N // NGROUP     # 5333
# b-gather sub-splits: (slot0, slot1, j0, j1) x 4 queues
GSUB = [(0, 1024, 0, 8), (1024, 2176, 8, 17),
        (2176, 3200, 17, 25), (3200, 4352, 25, 34)]

_CACHE = {}


def _k0(jj, ns_src, nj_src, ns_out, nb):
    kc = int(round((jj * P + P // 2) * ns_src / ns_out / P))
    return min(max(kc - 2, 0), nj_src - nb)


def _build_nc():
    import concourse.bacc as bacc
    import concourse.tile as tile
    import concourse.mybir as mybir

    f32 = mybir.dt.float32
    bf16 = mybir.dt.bfloat16
    fp8 = mybir.dt.float8e4
    i16 = mybir.dt.int16
    Alu = mybir.AluOpType
    Act = mybir.ActivationFunctionType
    Ax = mybir.AxisListType

    nc = bacc.Bacc(
        "TRN2", target_bir_lowering=False, debug=False, num_devices=N_CORES,
        num_swdge_queues=4,
    )

    # ---- I/O ----
    xT = nc.dram_tensor("xT", [IN, BC], f32, kind="ExternalInput")
    w1p = nc.dram_tensor("w1p", [P, NJ1 * 16], f32, kind="ExternalInput")
    w2p = nc.dram_tensor("w2p", [P, NJ2 * 16], f32, kind="ExternalInput")
    w3p = nc.dram_tensor("w3p", [P, NJ3 * 16], f32, kind="ExternalInput")
    oh1a = nc.dram_tensor("oh1a", [P, NJ1 * 2 * P], fp8, kind="ExternalInput")
    oh1b = nc.dram_tensor("oh1b", [P, NJ1 * 2 * P], fp8, kind="ExternalInput")
    oh2 = nc.dram_tensor("oh2", [P, NJ2 * NB2 * P], fp8, kind="ExternalInput")
    oh3 = nc.dram_tensor("oh3", [P, NJ3 * NB3 * P], fp8, kind="ExternalInput")
    i2d = nc.dram_tensor("i2", [P, NS2 // 16], i16, kind="ExternalInput")
    i3d = nc.dram_tensor("i3", [P, NS3 // 16], i16, kind="ExternalInput")
    gmask = nc.dram_tensor("gmask", [P, NJ3 * NGROUP], fp8, kind="ExternalInput")
    out_d = nc.dram_tensor("out", [1, NGROUP * BC], f32, kind="ExternalOutput")

    # collective buffers
    cin1 = [nc.dram_tensor(f"cin1_{k}", [P, JCH1 * BC], fp8, kind="Internal")
            for k in range(NCH)]
    cin2 = [nc.dram_tensor(f"cin2_{k}", [P, JCH2 * BC], fp8, kind="Internal")
            for k in range(NCH)]
    g1 = nc.dram_tensor("g1", [NCH * SH * P * JCH1, BC], fp8, kind="Internal")
    g2 = nc.dram_tensor("g2", [NCH * SH * P * JCH2, BC], fp8, kind="Internal")
    win = nc.dram_tensor("win", [P, 16], f32, kind="Internal")
    warm = nc.dram_tensor("warm", [SH * P, 16], f32, kind="Internal")
    pin = nc.dram_tensor("pin", [NGROUP, BC], f32, kind="Internal")
    pall = nc.dram_tensor("pall", [SH, NGROUP * BC], f32, kind="Internal")

    shard_groups = [[0, 1, 2, 3], [4, 5, 6, 7]]
    ROWS1 = SH * P * JCH1   # g1 rows per chunk
    ROWS2 = SH * P * JCH2

    with tile.TileContext(nc) as tc:
        with (
            tc.tile_pool(name="big", bufs=1) as big,
            tc.tile_pool(name="strm", bufs=2) as strm,
            tc.tile_pool(name="cmb", bufs=1) as cmb,
            tc.tile_pool(name="small", bufs=2) as small,
            tc.tile_pool(name="psum", bufs=1, space="PSUM") as psum,
        ):
            # ---- warm-up collective ----
            wsb = small.tile([P, 16], f32, tag="wsb")
            nc.vector.memset(wsb[:], 0.0)
            nc.sync.dma_start(win[:], wsb[:])
            nc.gpsimd.collective_compute(
                "AllGather", Alu.bypass, replica_groups=shard_groups,
                ins=[win[:]], outs=[warm[:]],
            )

            # ---- x load + quantize ----
            xf = big.tile([P, 2, BC], f32, tag="xf")
            nc.vector.memset(xf[:], 0.0)
            nc.sync.dma_start(xf[:, 0], xT[0:P, :])
            nc.sync.dma_start(xf[0 : IN - P, 1], xT[P:IN, :])
            xq = big.tile([P, 2, BC], fp8, tag="xq")
            nc.scalar.copy(xq[:], xf[:])

            # ---- idx loads ----
            i2sb = big.tile([P, NS2 // 16], i16, tag="i2sb")
            nc.sync.dma_start(i2sb[:], i2d[:])
            i3sb = big.tile([P, NS3 // 16], i16, tag="i3sb")
            nc.sync.dma_start(i3sb[:], i3d[:])
            b2 = big.tile([P, NJ2, BC], fp8, tag="b2")
            b3 = big.tile([P, NJ3, BC], fp8, tag="b3")
            cx = []
            for k_ in range(4):
                cxk = big.tile([P, NJ2, BC], fp8, tag=f"cx{k_}")
                cx.append(cxk)
            sems = [nc.alloc_semaphore(f"bdma{i}") for i in range(8)]

            # ---- coefficient prep ----
            def coeffs(wp, nj, lname):
                wt = strm.tile([P, nj * 16], f32, tag="wt")
                nc.sync.dma_start(wt[:], wp[:])
                e = strm.tile([P, nj * 16], f32, tag="e")
                nc.scalar.activation(e[:], wt[:], Act.Exp)
                e3 = e[:].rearrange("p (j g) -> p j g", g=16)
                e4 = e[:].rearrange("p (j h q) -> p j h q", h=4, q=4)
                ssum = small.tile([P, nj], f32, tag="ss")
                nc.vector.reduce_sum(ssum[:], e3, axis=Ax.X)
                r = small.tile([P, nj], f32, tag="r")
                nc.vector.reciprocal(r[:], ssum[:])
                c0 = small.tile([P, nj], f32, tag=f"c0{lname}")
                c1 = small.tile([P, nj], f32, tag=f"c1{lname}")
                c2 = small.tile([P, nj], f32, tag=f"c2{lname}")
                c3 = small.tile([P, nj], f32, tag=f"c3{lname}")
                nc.vector.reduce_sum(c0[:], e4[:, :, 2:4, :], axis=Ax.XY)
                t1 = small.tile([P, nj], f32, tag="t1")
                t2 = small.tile([P, nj], f32, tag="t2")
                nc.vector.reduce_sum(t1[:], e4[:, :, 0:2, 2:4], axis=Ax.XY)
                nc.vector.reduce_sum(t2[:], e4[:, :, 2:4, 0:2], axis=Ax.XY)
                nc.vector.tensor_sub(c1[:], t1[:], t2[:])
                nc.vector.reduce_sum(t1[:], e4[:, :, 1, :], axis=Ax.X)
                nc.vector.reduce_sum(t2[:], e4[:, :, 2, :], axis=Ax.X)
                nc.vector.tensor_sub(c2[:], t1[:], t2[:])
                f = small.tile([P, nj, 7], f32, tag="f")
                nc.vector.tensor_sub(f[:], e3[:, :, 1:8], e3[:, :, 14:7:-1])
                u1 = small.tile([P, nj], f32, tag="u1")
                u2 = small.tile([P, nj], f32, tag="u2")
                nc.vector.tensor_sub(u1[:], f[:, :, 0], f[:, :, 1])
                nc.vector.tensor_add(u2[:], f[:, :, 3], f[:, :, 6])
                nc.vector.tensor_sub(u1[:], u1[:], u2[:])
                nc.vector.scalar_tensor_tensor(
                    c3[:], f[:, :, 5], -2.0, u1[:], op0=Alu.mult, op1=Alu.add
                )
                for ck in (c0, c1, c2, c3):
                    nc.vector.tensor_mul(ck[:], ck[:], r[:])
                # expand to per-element tiles by log-doubling (plain strided
                # copies on DVE; runs during idle collective windows)
                for k_, ck in enumerate((c0, c1, c2, c3)):
                    xk = cx[k_]
                    nc.vector.tensor_scalar_mul(
                        xk[:, :nj, 0:1], ck[:].unsqueeze(2), 1.0)
                    w = 1
                    while w < BC:
                        nc.vector.tensor_scalar_mul(
                            xk[:, :nj, w : 2 * w], xk[:, :nj, 0:w], 1.0)
                        w *= 2
                return c0, c1, c2, c3

            # ---- combine: h = C0 + C1 a + C2 b + C3 ab, all-fp8 DVE ----
            JMAX = max(JCH1, JCH2, JCH3)

            def combine(h, a_sb, b_sb, cs, j0, j1, u_on_act):
                nj = j1 - j0
                tf = cmb.tile([P, JMAX, BC], fp8, tag="ct")
                uf = cmb.tile([P, JMAX, BC], fp8, tag="cu")
                vf = cmb.tile([P, JMAX, BC], fp8, tag="cv")
                t, u, v = tf[:, :nj], uf[:, :nj], vf[:, :nj]
                av = a_sb[:, j0:j1]
                bv = b_sb[:, j0:j1]
                c0x, c1x, c2x, c3x = (c[:, j0:j1] for c in cx)
                nc.vector.tensor_mul(t, av, bv)
                nc.vector.tensor_mul(t, t, c3x)
                nc.vector.tensor_mul(v, bv, c2x)
                nc.vector.tensor_add(t, t, v)
                nc.vector.tensor_mul(u, av, c1x)
                nc.vector.tensor_add(u, u, c0x)
                nc.vector.tensor_add(h[:, j0:j1], t, u)

            GJ = 6  # j-cols per PSUM group

            def jgroups(lo, hi):
                return [(g0, min(g0 + GJ, hi)) for g0 in range(lo, hi, GJ)]

            # ================= LAYER 1 =================
            cs1 = coeffs(w1p, NJ1, "a")
            hA_ = big.tile([P, NJ3, BC], fp8, tag="hA")
            h1 = hA_[:, :NJ1]
            aL_ = big.tile([P, NJ2, BC], fp8, tag="aL")
            a1 = aL_[:, :NJ1]
            b1 = big.tile([P, NJ1, BC], fp8, tag="b1")
            gm = big.tile([P, NJ3, NGROUP], fp8, tag="gm")
            nc.sync.dma_start(
                gm[:], gmask[:].rearrange("p (j g) -> p j g", g=NGROUP)
            )

            # per-chunk: PE groups then combine then AllGather
            for ch in range(NCH):
                j0c, j1c = ch * JCH1, (ch + 1) * JCH1
                for g0, ge in jgroups(j0c, j1c):
                    gw = ge - g0
                    oha = strm.tile([P, GJ * 2 * P], fp8, tag="oha")
                    ohb = strm.tile([P, GJ * 2 * P], fp8, tag="ohb")
                    nc.sync.dma_start(
                        oha[:, : gw * 2 * P], oh1a[:, g0 * 2 * P : ge * 2 * P])
                    nc.sync.dma_start(
                        ohb[:, : gw * 2 * P], oh1b[:, g0 * 2 * P : ge * 2 * P])
                    pa = psum.tile([P, GJ, BC], f32, tag="pa")
                    pb = psum.tile([P, GJ, BC], f32, tag="pb")
                    for jl in range(gw):
                        for k in range(2):
                            nc.tensor.matmul(
                                pa[:, jl], oha[:, (jl * 2 + k) * P : (jl * 2 + k + 1) * P],
                                xq[:, k], start=(k == 0), stop=(k == 1),
                            )
                        for k in range(2):
                            nc.tensor.matmul(
                                pb[:, jl], ohb[:, (jl * 2 + k) * P : (jl * 2 + k + 1) * P],
                                xq[:, k], start=(k == 0), stop=(k == 1),
                            )
                    nc.scalar.copy(a1[:, g0:ge], pa[:, :gw])
                    nc.scalar.copy(b1[:, g0:ge], pb[:, :gw])
                combine(h1, a1, b1, cs1, j0c, j1c, u_on_act=True)
                nc.sync.dma_start(
                    cin1[ch][:], h1[:, j0c:j1c].rearrange("p j b -> p (j b)"))
                nc.gpsimd.collective_compute(
                    "AllGather", Alu.bypass, replica_groups=shard_groups,
                    ins=[cin1[ch][:]],
                    outs=[g1[ch * ROWS1 : (ch + 1) * ROWS1, :]],
                )

            # prep b2 sub-gathers (desc-gen runs early; fired after AG)
            for q, (s0, s1, j0, j1) in enumerate(GSUB):
                nc.gpsimd.dma_gather(
                    b2[:, j0:j1], g1[:], i2sb[:, s0 // 16 : s1 // 16],
                    s1 - s0, s1 - s0, BC,
                    prepare_only=True, sem=sems[q], queue_num=q,
                    single_packet=False,
                )

            def pe_a_gather(nj, nb, oh_d, src, ns_src, nj_src, ns_out, a_sb):
                for gi, (g0, ge) in enumerate(jgroups(0, nj)):
                    gw = ge - g0
                    ohg = strm.tile([P, GJ * NB3 * P], fp8, tag="ohg")
                    nc.sync.dma_start(
                        ohg[:, : gw * nb * P],
                        oh_d[:, g0 * nb * P : ge * nb * P],
                    )
                    pa = psum.tile(
                        [P, GJ, BC], f32, tag="pa" if gi % 2 == 0 else "pb")
                    for jl in range(gw):
                        jj = g0 + jl
                        k0 = _k0(jj, ns_src, nj_src, ns_out, nb)
                        for v in range(nb):
                            nc.tensor.matmul(
                                pa[:, jl],
                                ohg[:, (jl * nb + v) * P : (jl * nb + v + 1) * P],
                                src[:, k0 + v],
                                start=(v == 0), stop=(v == nb - 1),
                            )
                    nc.scalar.copy(a_sb[:, g0:ge], pa[:, :gw])

            # ================= LAYER 2 =================
            cs2 = coeffs(w2p, NJ2, "b")
            h2 = big.tile([P, NJ2, BC], fp8, tag="h2")
            a2 = big.tile([P, NJ2, BC], fp8, tag="aL")
            pe_a_gather(NJ2, NB2, oh2, h1, NS1, NJ1, NS2, a2)

            for q in range(4):
                nc.gpsimd.trigger_dma(count=None, queue_num=q)

            for ch in range(NCH):
                j0c, j1c = ch * JCH2, (ch + 1) * JCH2
                combine(h2, a2, b2, cs2, j0c, j1c, u_on_act=True)
                nc.sync.dma_start(
                    cin2[ch][:], h2[:, j0c:j1c].rearrange("p j b -> p (j b)"))
                nc.gpsimd.collective_compute(
                    "AllGather", Alu.bypass, replica_groups=shard_groups,
                    ins=[cin2[ch][:]],
                    outs=[g2[ch * ROWS2 : (ch + 1) * ROWS2, :]],
                )

            for q, (s0, s1, j0, j1) in enumerate(GSUB):
                nc.gpsimd.dma_gather(
                    b3[:, j0:j1], g2[:], i3sb[:, s0 // 16 : s1 // 16],
                    s1 - s0, s1 - s0, BC,
                    prepare_only=True, sem=sems[4 + q], queue_num=q,
                    single_packet=False,
                )

            # ================= LAYER 3 =================
            cs3 = coeffs(w3p, NJ3, "a")
            h3 = big.tile([P, NJ3, BC], fp8, tag="hA")
            a3 = big.tile([P, NJ3, BC], fp8, tag="aL")
            pe_a_gather(NJ3, NB3, oh3, h2, NS2, NJ2, NS3, a3)

            for q in range(4):
                nc.gpsimd.trigger_dma(count=None, queue_num=q)

            pg = psum.tile([NGROUP, BC], f32, tag="pg")
            for ch in range(NCH):
                j0c, j1c = ch * JCH3, (ch + 1) * JCH3
                combine(h3, a3, b3, cs3, j0c, j1c, u_on_act=True)
                for j in range(j0c, j1c):
                    nc.tensor.matmul(
                        pg[:], gm[:, j], h3[:, j],
                        start=(j == 0), stop=(j == NJ3 - 1),
                    )

            # ---- cross-shard reduction + output ----
            psc = big.tile([NGROUP, BC], f32, tag="psc")
            nc.scalar.copy(psc[:], pg[:])
            nc.sync.dma_start(pin[:], psc[:])
            nc.gpsimd.collective_compute(
                "AllGather", Alu.bypass, replica_groups=shard_groups,
                ins=[pin[:]], outs=[pall[:]],
            )
            pall_sb = big.tile([1, SH, NGROUP * BC], f32, tag="pall_sb")
            nc.sync.dma_start(pall_sb[:], pall[:].rearrange("s u -> (s u)"))
            osb = big.tile([1, NGROUP * BC], f32, tag="osb")
            nc.vector.tensor_add(osb[:], pall_sb[:, 0], pall_sb[:, 1])
            nc.vector.tensor_add(osb[:], osb[:], pall_sb[:, 2])
            nc.vector.tensor_add(osb[:], osb[:], pall_sb[:, 3])
            nc.scalar.mul(osb[:], osb[:], 1.0 / TAU)
            # consume the warm-up collective output so DCE keeps it
            wsb2 = small.tile([1, 16], f32, tag="wsb2")
            nc.sync.dma_start(wsb2[:], warm[0:1, :])
            nc.vector.tensor_add(osb[:, :16], osb[:, :16], wsb2[:])
            nc.sync.dma_start(out_d[:], osb[:])

    # ---- surgical sync rewiring for the prepare_only gathers ----
    # Tile places the g-buffer RAW dep (Collectives sem) on the prep itself
    # (blocking desc-gen until the AllGather lands) and leaves consumers
    # waiting on a DMASW lane sem that nothing increments (the SDMA completion
    # bumps the sem= baked into the descriptors instead).  Move the
    # Collectives waits prep -> trigger and repoint the orphaned DMASW waits
    # at the descriptor completion sems.
    import bass_rust

    insts = list(nc.all_instructions())
    preps = [
        i for i in insts
        if type(i).__name__ == "InstDMAGatherAnt" and getattr(i, "gen_mode", 0) == 1
    ]
    assert len(preps) == 8, len(preps)
    # triggers in program order, paired per queue with preps in order
    trig_by_q = {}
    for i in insts:
        if type(i).__name__ == "InstTriggerDma":
            trig_by_q.setdefault(i.queue_num, []).append(i)
    prep_by_q = {}
    for p in preps:
        prep_by_q.setdefault(p.queue_num, []).append(p)
    updated = set()
    for i in insts:
        if i.sync_info:
            for u_ in (i.sync_info.on_update or []):
                updated.add(u_.ant_name)
    lane_sem = {}
    for li, p in enumerate(preps):
        lane_sem[f"DMASW{li % 8}"] = p.sync_info.on_update[0]
    for q, plist in prep_by_q.items():
        tlist = trig_by_q[q]
        assert len(plist) == len(tlist), (q, len(plist), len(tlist))
        for p, t in zip(plist, tlist):
            keep, move = [], []
            for w in p.sync_info.on_wait or []:
                (move if w.ant_name.startswith("Collectives") else keep).append(w)
            p.sync_info.on_wait = keep
            t.sync_info.on_wait = list(t.sync_info.on_wait or []) + move
    n_rewired = 0
    for i in insts:
        si = i.sync_info
        if not si or not si.on_wait:
            continue
        nw, changed = [], False
        for w in si.on_wait:
            base = w.ant_name.rsplit("_", 1)[0]
            if (
                w.ant_name.startswith("DMASW")
                and w.ant_name not in updated
                and base in lane_sem
            ):
                u_ = lane_sem[base]
                nw.append(bass_rust.SyncWait(
                    sync_type="semaphore", id=u_.id, ant_name=u_.ant_name,
                    wait_mode=w.wait_mode, wait_value=w.wait_value, wait_reg=None,
                ))
                changed = True
                n_rewired += 1
            else:
                nw.append(w)
        if changed:
            si.on_wait = nw
    assert n_rewired >= 8, f"expected orphan DMASW waits, rewired {n_rewired}"

    nc.compile()
    return nc


# ===================== host packing =====================

def _wrap_idx(ii):
    w = ii.astype(np.int16).reshape(-1, 16).T
    return np.ascontiguousarray(np.tile(w, (8, 1)))


def _pack_w(w_eff, nj):
    # slot t = j*128 + p  ->  packed[p, j*16+g]
    return np.ascontiguousarray(
        w_eff.reshape(nj, P, 16).transpose(1, 0, 2).reshape(P, nj * 16)
    )


PAD_ROW = np.full(16, -20.0, dtype=np.float32)
PAD_ROW[0] = 20.0  # softmax -> ~one-hot FALSE gate -> h = 0


def _assign_slots(la, ns_out):
    """Sort outputs by local a-slot, spread pads uniformly."""
    n = len(la)
    assert n <= ns_out, f"shard overflow: {n} > {ns_out}"
    order = np.argsort(la, kind="stable")
    slots = np.empty(n, dtype=np.int64)
    slots[order] = (np.arange(n, dtype=np.int64) * ns_out) // n
    return slots


def _onehot_pack(la_by_slot, ns_src, nj_src, ns_out, nb):
    """[P, NT*nb*P] uint8 one-hot lhsT data (0x00 / 0x38==1.0 e4m3)."""
    nt = ns_out // P
    oh = np.zeros((P, nt * nb * P), dtype=np.uint8)
    for jj in range(nt):
        k0 = _k0(jj, ns_src, nj_src, ns_out, nb)
        for m in range(P):
            la = la_by_slot[jj * P + m]
            if la < 0:
                continue
            r = la - k0 * P
            v, rr = divmod(r, P)
            assert 0 <= v < nb, f"window violation jj={jj} m={m} la={la} k0={k0}"
            oh[rr, (jj * nb + v) * P + m] = 0x38
    return oh.view(FP8)


def _host_pack(inputs):
    x = np.asarray(inputs["x"], dtype=np.float32)
    w1 = np.asarray(inputs["w1"], dtype=np.float32)
    w2 = np.asarray(inputs["w2"], dtype=np.float32)
    w3 = np.asarray(inputs["w3"], dtype=np.float32)
    i1a = np.asarray(inputs["idx1a"]).astype(np.int64)
    i1b = np.asarray(inputs["idx1b"]).astype(np.int64)
    i2a = np.asarray(inputs["idx2a"]).astype(np.int64)
    i2b = np.asarray(inputs["idx2b"]).astype(np.int64)
    i3a = np.asarray(inputs["idx3a"]).astype(np.int64)
    i3b = np.asarray(inputs["idx3b"]).astype(np.int64)

    per_shard = [dict() for _ in range(SH)]

    # ---------- layer 1: contiguous assignment ----------
    for s in range(SH):
        sel = np.arange(s * R1, (s + 1) * R1)
        w_eff = np.concatenate(
            [w1[sel], np.tile(PAD_ROW, (NS1 - R1, 1))], axis=0
        )
        per_shard[s]["w1p"] = _pack_w(w_eff, NJ1)
        for nm, idx in (("oh1a", i1a), ("oh1b", i1b)):
            oh = np.zeros((P, NJ1 * 2 * P), dtype=np.uint8)
            t = np.arange(R1)
            jj, m = t // P, t % P
            src = idx[s * R1 + t]
            k, rr = src // P, src % P
            oh[rr, (jj * 2 + k) * P + m] = 0x38
            per_shard[s][nm] = oh.view(FP8)

    # ---------- layer 2 ----------
    s2 = i2a // R1
    la2 = i2a - s2 * R1
    slot2 = np.zeros(L1N, dtype=np.int64)
    for s in range(SH):
        sel = np.where(s2 == s)[0]
        sl = _assign_slots(la2[sel], NS2)
        slot2[sel] = s * NS2 + sl
        la_by_slot = np.full(NS2, -1, dtype=np.int64)
        la_by_slot[sl] = la2[sel]
        per_shard[s]["oh2"] = _onehot_pack(la_by_slot, NS1, NJ1, NS2, NB2)
        w_eff = np.tile(PAD_ROW, (NS2, 1))
        w_eff[sl] = w2[sel]
        per_shard[s]["w2p"] = _pack_w(w_eff, NJ2)
        ib = i2b[sel]
        sb, tb = ib // R1, ib % R1
        jb, pb = tb // P, tb % P
        ch = jb // JCH1
        row = ((ch * SH + sb) * P + pb) * JCH1 + (jb - ch * JCH1)
        idx_eff = np.zeros(NS2, dtype=np.int64)
        idx_eff[sl] = row
        per_shard[s]["i2"] = _wrap_idx(idx_eff)

    # ---------- layer 3 ----------
    g3 = slot2[i3a]
    s3 = g3 // NS2
    la3 = g3 - s3 * NS2
    grp = np.arange(L3N) // SPG
    for s in range(SH):
        sel = np.where(s3 == s)[0]
        sl = _assign_slots(la3[sel], NS3)
        la_by_slot = np.full(NS3, -1, dtype=np.int64)
        la_by_slot[sl] = la3[sel]
        per_shard[s]["oh3"] = _onehot_pack(la_by_slot, NS2, NJ2, NS3, NB3)
        w_eff = np.tile(PAD_ROW, (NS3, 1))
        w_eff[sl] = w3[sel]
        per_shard[s]["w3p"] = _pack_w(w_eff, NJ3)
        gb = slot2[i3b[sel]]
        sb, tb = gb // NS2, gb % NS2
        jb, pb = tb // P, tb % P
        ch = jb // JCH2
        row = ((ch * SH + sb) * P + pb) * JCH2 + (jb - ch * JCH2)
        idx_eff = np.zeros(NS3, dtype=np.int64)
        idx_eff[sl] = row
        per_shard[s]["i3"] = _wrap_idx(idx_eff)
        gmask = np.zeros((NS3, NGROUP), dtype=np.uint8)
        gmask[sl, grp[sel]] = 0x38
        per_shard[s]["gmask"] = np.ascontiguousarray(
            gmask.reshape(NJ3, P, NGROUP).transpose(1, 0, 2).reshape(P, NJ3 * NGROUP)
        ).view(FP8)

    in_maps = []
    for c in range(N_CORES):
        G, s = c // SH, c % SH
        m_ = dict(per_shard[s])
        m_["xT"] = np.ascontiguousarray(x[G * BC : (G + 1) * BC].T)
        in_maps.append(m_)
    return in_maps


LAST_RESULTS = None


def kernel(**inputs):
    global LAST_RESULTS
    from concourse.bass_utils import run_bass_kernel_spmd

    if "nc" not in _CACHE:
        _CACHE["nc"] = _build_nc()
    nc = _CACHE["nc"]

    in_maps = _host_pack(inputs)
    trace = bool(int(os.environ.get("KERNEL_TRACE", "0")))
    res = run_bass_kernel_spmd(
        nc, in_maps, core_ids=list(range(N_CORES)), trace=trace
    )
    LAST_RESULTS = res

    out = np.empty((B, NGROUP), dtype=np.float32)
    for g_ in range(BG):
        rc = res.results[g_ * SH]["out"].reshape(NGROUP, BC)
        out[g_ * BC : (g_ + 1) * BC, :] = rc.T
    return out


# revision 27
# speedup vs baseline: 1.0980x; 1.0980x over previous
"""Trainium2 Bass kernel for a 3-layer difflogic network (nn_Net_48610439856713).

Math: each layer o computes softmax(w[o])·ops16(a, b) with a = h[:, ia[o]],
b = h[:, ib[o]].  The 16 relaxed logic gates are affine in {1, a, b, ab},
so a layer is h' = C0 + C1·a + C2·b + C3·a·b with per-neuron coefficients
derived on-device from softmax(w).

Design (v3):
 - 8 cores = 2 batch groups x 4 neuron shards.  Activations and combine
   arithmetic in fp8(e4m3): the final GroupSum averages ~5333 neurons, so
   quantization noise cancels (validated ~7e-3 final rel err vs 2e-2 budget).
 - a-stream gathers run on the Tensor engine as one-hot matmuls: each layer's
   outputs are assigned to the core that owns their a-source shard and sorted
   by source slot, so each 128-output tile reads only NB source j-columns
   (3 for layer 2, 4 for layer 3).  One-hots are host-packed fp8 {0,1} data.
   Layer 1 (source x, 193 rows = 2 K-blocks) runs both streams on PE.
 - b-stream gathers (random remote rows) use SWDGE dma_gather from the
   AllGathered fp8 buffer.  Descriptor generation is hoisted off the critical
   path with prepare_only + trigger_dma; each layer's gather is split into
   4 sub-gathers on 4 SWDGE queues (the drain is latency-bound on random
   256B HBM reads, and parallel queue rings multiply outstanding requests).
 - Combine: u = C1*a + C0 runs per-j on the Scalar engine (per-partition
   scale/bias APs); the remaining 5 passes are full-chunk fp8 DVE ops
   (t = ab, t *= C3, v = b*C2, t += v, h = t + u), with the C3/C2 factors
   as stride-0 broadcast APs.
 - GroupSum = 0/1-mask matmuls on PE into PSUM (interleaved with the layer-3
   combine), then a tiny AllGather and partition-0 adds.

Host-side work is integer/layout only: slot permutations, one-hot placement
(bytes 0x00/0x38), index relabeling, int16 wrapping, weight-row packing.
All float arithmetic (softmax, combine, sums) runs on device.
"""

import os
import numpy as np
import ml_dtypes

FP8 = ml_dtypes.float8_e4m3fn

P = 128
B = 512
BG = 2                  # batch groups
SH = 4                  # neuron shards
BC = B // BG            # 256 batch per core
IN = 193
NGROUP = 3
TAU = 100.0
N_CORES = 8
NB2 = 3                 # source j-blocks per output tile, layer 2
NB3 = 4                 # layer 3

NJ1, R1 = 32, 4000      # layer1: 4000 real rows/shard, 4096 slots
NS1 = NJ1 * P
NJ2 = 34
NS2 = NJ2 * P           # 4352 slots (binomial 4000+-55, 6.4 sigma margin)
NJ3 = 34
NS3 = NJ3 * P
NCH = 2                 # AllGather chunks per layer
JCH1, JCH2, JCH3 = NJ1 // NCH, NJ2 // NCH, NJ3 // NCH
L1N = 16000
L3N = 15999
SPG = L3N // NGROUP     # 5333
# b-gather sub-splits: (slot0, slot1, j0, j1) x 4 queues
GSUB = [(0, 1024, 0, 8), (1024, 2176, 8, 17),
        (2176, 3200, 17, 25), (3200, 4352, 25, 34)]

_CACHE = {}


def _k0(jj, ns_src, nj_src, ns_out, nb):
    kc = int(round((jj * P + P // 2) * ns_src / ns_out / P))
    return min(max(kc - 2, 0), nj_src - nb)


def _build_nc():
    import concourse.bacc as bacc
    import concourse.tile as tile
    import concourse.mybir as mybir

    f32 = mybir.dt.float32
    bf16 = mybir.dt.bfloat16
    fp8 = mybir.dt.float8e4
    i16 = mybir.dt.int16
    Alu = mybir.AluOpType
    Act = mybir.ActivationFunctionType
    Ax = mybir.AxisListType

    nc = bacc.Bacc(
        "TRN2", target_bir_lowering=False, debug=False, num_devices=N_CORES,
        num_swdge_queues=4,
    )

    # ---- I/O ----
    xT = nc.dram_tensor("xT", [IN, BC], f32, kind="ExternalInput")
    w1p = nc.dram_tensor("w1p", [P, NJ1 * 16], f32, kind="ExternalInput")
    w2p = nc.dram_tensor("w2p", [P, NJ2 * 16], f32, kind="ExternalInput")
    w3p = nc.dram_tensor("w3p", [P, NJ3 * 16], f32, kind="ExternalInput")
    oh1a = nc.dram_tensor("oh1a", [P, NJ1 * 2 * P], fp8, kind="ExternalInput")
    oh1b = nc.dram_tensor("oh1b", [P, NJ1 * 2 * P], fp8, kind="ExternalInput")
    oh2 = nc.dram_tensor("oh2", [P, NJ2 * NB2 * P], fp8, kind="ExternalInput")
    oh3 = nc.dram_tensor("oh3", [P, NJ3 * NB3 * P], fp8, kind="ExternalInput")
    i2d = nc.dram_tensor("i2", [P, NS2 // 16], i16, kind="ExternalInput")
    i3d = nc.dram_tensor("i3", [P, NS3 // 16], i16, kind="ExternalInput")
    gmask = nc.dram_tensor("gmask", [P, NJ3 * NGROUP], fp8, kind="ExternalInput")
    out_d = nc.dram_tensor("out", [1, NGROUP * BC], f32, kind="ExternalOutput")

    # collective buffers
    cin1 = [nc.dram_tensor(f"cin1_{k}", [P, JCH1 * BC], fp8, kind="Internal")
            for k in range(NCH)]
    cin2 = [nc.dram_tensor(f"cin2_{k}", [P, JCH2 * BC], fp8, kind="Internal")
            for k in range(NCH)]
    g1 = nc.dram_tensor("g1", [NCH * SH * P * JCH1, BC], fp8, kind="Internal")
    g2 = nc.dram_tensor("g2", [NCH * SH * P * JCH2, BC], fp8, kind="Internal")
    win = nc.dram_tensor("win", [P, 16], f32, kind="Internal")
    warm = nc.dram_tensor("warm", [SH * P, 16], f32, kind="Internal")
    pin = nc.dram_tensor("pin", [NGROUP, BC], f32, kind="Internal")
    pall = nc.dram_tensor("pall", [SH, NGROUP * BC], f32, kind="Internal")

    shard_groups = [[0, 1, 2, 3], [4, 5, 6, 7]]
    ROWS1 = SH * P * JCH1   # g1 rows per chunk
    ROWS2 = SH * P * JCH2

    with tile.TileContext(nc) as tc:
        with (
            tc.tile_pool(name="big", bufs=1) as big,
            tc.tile_pool(name="strm", bufs=2) as strm,
            tc.tile_pool(name="cmb", bufs=1) as cmb,
            tc.tile_pool(name="small", bufs=2) as small,
            tc.tile_pool(name="psum", bufs=1, space="PSUM") as psum,
        ):
            # ---- warm-up collective ----
            wsb = small.tile([P, 16], f32, tag="wsb")
            nc.vector.memset(wsb[:], 0.0)
            nc.sync.dma_start(win[:], wsb[:])
            nc.gpsimd.collective_compute(
                "AllGather", Alu.bypass, replica_groups=shard_groups,
                ins=[win[:]], outs=[warm[:]],
            )

            # ---- x load + quantize ----
            xf = big.tile([P, 2, BC], f32, tag="xf")
            nc.vector.memset(xf[:], 0.0)
            nc.sync.dma_start(xf[:, 0], xT[0:P, :])
            nc.sync.dma_start(xf[0 : IN - P, 1], xT[P:IN, :])
            xq = big.tile([P, 2, BC], fp8, tag="xq")
            nc.scalar.copy(xq[:], xf[:])

            # ---- idx loads ----
            i2sb = big.tile([P, NS2 // 16], i16, tag="i2sb")
            nc.sync.dma_start(i2sb[:], i2d[:])
            i3sb = big.tile([P, NS3 // 16], i16, tag="i3sb")
            nc.sync.dma_start(i3sb[:], i3d[:])
            b2 = big.tile([P, NJ2, BC], fp8, tag="b2")
            b3 = big.tile([P, NJ3, BC], fp8, tag="b3")
            sems = [nc.alloc_semaphore(f"bdma{i}") for i in range(8)]

            # ---- coefficient prep ----
            def coeffs(wp, nj, lname):
                wt = strm.tile([P, nj * 16], f32, tag="wt")
                nc.sync.dma_start(wt[:], wp[:])
                e = strm.tile([P, nj * 16], f32, tag="e")
                nc.scalar.activation(e[:], wt[:], Act.Exp)
                e3 = e[:].rearrange("p (j g) -> p j g", g=16)
                e4 = e[:].rearrange("p (j h q) -> p j h q", h=4, q=4)
                ssum = small.tile([P, nj], f32, tag="ss")
                nc.vector.reduce_sum(ssum[:], e3, axis=Ax.X)
                r = small.tile([P, nj], f32, tag="r")
                nc.vector.reciprocal(r[:], ssum[:])
                c0 = small.tile([P, nj], f32, tag=f"c0{lname}")
                c1 = small.tile([P, nj], f32, tag=f"c1{lname}")
                c2 = small.tile([P, nj], f32, tag=f"c2{lname}")
                c3 = small.tile([P, nj], f32, tag=f"c3{lname}")
                nc.vector.reduce_sum(c0[:], e4[:, :, 2:4, :], axis=Ax.XY)
                t1 = small.tile([P, nj], f32, tag="t1")
                t2 = small.tile([P, nj], f32, tag="t2")
                nc.vector.reduce_sum(t1[:], e4[:, :, 0:2, 2:4], axis=Ax.XY)
                nc.vector.reduce_sum(t2[:], e4[:, :, 2:4, 0:2], axis=Ax.XY)
                nc.vector.tensor_sub(c1[:], t1[:], t2[:])
                nc.vector.reduce_sum(t1[:], e4[:, :, 1, :], axis=Ax.X)
                nc.vector.reduce_sum(t2[:], e4[:, :, 2, :], axis=Ax.X)
                nc.vector.tensor_sub(c2[:], t1[:], t2[:])
                f = small.tile([P, nj, 7], f32, tag="f")
                nc.vector.tensor_sub(f[:], e3[:, :, 1:8], e3[:, :, 14:7:-1])
                u1 = small.tile([P, nj], f32, tag="u1")
                u2 = small.tile([P, nj], f32, tag="u2")
                nc.vector.tensor_sub(u1[:], f[:, :, 0], f[:, :, 1])
                nc.vector.tensor_add(u2[:], f[:, :, 3], f[:, :, 6])
                nc.vector.tensor_sub(u1[:], u1[:], u2[:])
                nc.vector.scalar_tensor_tensor(
                    c3[:], f[:, :, 5], -2.0, u1[:], op0=Alu.mult, op1=Alu.add
                )
                for ck in (c0, c1, c2, c3):
                    nc.vector.tensor_mul(ck[:], ck[:], r[:])
                return c0, c1, c2, c3

            # ---- combine: h = C0 + C1 a + C2 b + C3 ab, all-fp8 DVE ----
            JMAX = max(JCH1, JCH2, JCH3)

            def combine(h, a_sb, b_sb, cs, j0, j1, u_on_act):
                c0, c1, c2, c3 = cs
                nj = j1 - j0
                tf = cmb.tile([P, JMAX, BC], fp8, tag="ct")
                uf = cmb.tile([P, JMAX, BC], fp8, tag="cu")
                vf = cmb.tile([P, JMAX, BC], fp8, tag="cv")
                t, u, v = tf[:, :nj], uf[:, :nj], vf[:, :nj]
                av = a_sb[:, j0:j1]
                bv = b_sb[:, j0:j1]

                def cb(c):
                    return c[:, j0:j1].unsqueeze(2).broadcast_to([P, nj, BC])

                if u_on_act:
                    # u = C1 a + C0 per-j on the Scalar engine
                    for jl in range(nj):
                        j = j0 + jl
                        nc.scalar.activation(
                            u[:, jl], a_sb[:, j], Act.Identity,
                            bias=c0[:, j : j + 1], scale=c1[:, j : j + 1],
                        )
                else:
                    nc.vector.tensor_mul(u, av, cb(c1))
                    nc.vector.tensor_add(u, u, cb(c0))
                nc.vector.tensor_mul(t, av, bv)
                nc.vector.tensor_mul(t, t, cb(c3))
                nc.vector.tensor_mul(v, bv, cb(c2))
                nc.vector.tensor_add(t, t, v)
                nc.vector.tensor_add(h[:, j0:j1], t, u)

            # w = a * C3 has no dependency on the gathered b-stream, so both
            # chunks' w-passes are emitted first and run on the DVE during the
            # otherwise-idle b-gather drain window.
            def combine_w(a_sb, cs, j0, j1, ch):
                c0, c1, c2, c3 = cs
                nj = j1 - j0
                wf = cmb.tile([P, JMAX, BC], fp8, tag=f"cw{ch}")
                w = wf[:, :nj]
                nc.vector.tensor_mul(
                    w, a_sb[:, j0:j1],
                    c3[:, j0:j1].unsqueeze(2).broadcast_to([P, nj, BC]))
                return w

            def combine_rest(h, w, a_sb, b_sb, cs, j0, j1):
                c0, c1, c2, c3 = cs
                nj = j1 - j0
                tf = cmb.tile([P, JMAX, BC], fp8, tag="ct")
                uf = cmb.tile([P, JMAX, BC], fp8, tag="cu")
                vf = cmb.tile([P, JMAX, BC], fp8, tag="cv")
                t, u, v = tf[:, :nj], uf[:, :nj], vf[:, :nj]
                bv = b_sb[:, j0:j1]
                for jl in range(nj):
                    j = j0 + jl
                    nc.scalar.activation(
                        u[:, jl], a_sb[:, j], Act.Identity,
                        bias=c0[:, j : j + 1], scale=c1[:, j : j + 1],
                    )
                nc.vector.tensor_mul(t, w, bv)           # (a C3) b
                nc.vector.tensor_mul(
                    v, bv,
                    c2[:, j0:j1].unsqueeze(2).broadcast_to([P, nj, BC]))
                nc.vector.tensor_add(t, t, v)
                nc.vector.tensor_add(h[:, j0:j1], t, u)

            GJ = 6  # j-cols per PSUM group

            def jgroups(lo, hi):
                return [(g0, min(g0 + GJ, hi)) for g0 in range(lo, hi, GJ)]

            # ================= LAYER 1 =================
            cs1 = coeffs(w1p, NJ1, "a")
            hA_ = big.tile([P, NJ3, BC], fp8, tag="hA")
            h1 = hA_[:, :NJ1]
            aL_ = big.tile([P, NJ2, BC], fp8, tag="aL")
            a1 = aL_[:, :NJ1]
            b1 = big.tile([P, NJ1, BC], fp8, tag="b1")
            gm = big.tile([P, NJ3, NGROUP], fp8, tag="gm")
            nc.sync.dma_start(
                gm[:], gmask[:].rearrange("p (j g) -> p j g", g=NGROUP)
            )

            # per-chunk: PE groups then combine then AllGather
            for ch in range(NCH):
                j0c, j1c = ch * JCH1, (ch + 1) * JCH1
                for g0, ge in jgroups(j0c, j1c):
                    gw = ge - g0
                    oha = strm.tile([P, GJ * 2 * P], fp8, tag="oha")
                    ohb = strm.tile([P, GJ * 2 * P], fp8, tag="ohb")
                    nc.sync.dma_start(
                        oha[:, : gw * 2 * P], oh1a[:, g0 * 2 * P : ge * 2 * P])
                    nc.sync.dma_start(
                        ohb[:, : gw * 2 * P], oh1b[:, g0 * 2 * P : ge * 2 * P])
                    pa = psum.tile([P, GJ, BC], f32, tag="pa")
                    pb = psum.tile([P, GJ, BC], f32, tag="pb")
                    for jl in range(gw):
                        for k in range(2):
                            nc.tensor.matmul(
                                pa[:, jl], oha[:, (jl * 2 + k) * P : (jl * 2 + k + 1) * P],
                                xq[:, k], start=(k == 0), stop=(k == 1),
                            )
                        for k in range(2):
                            nc.tensor.matmul(
                                pb[:, jl], ohb[:, (jl * 2 + k) * P : (jl * 2 + k + 1) * P],
                                xq[:, k], start=(k == 0), stop=(k == 1),
                            )
                    nc.scalar.copy(a1[:, g0:ge], pa[:, :gw])
                    nc.scalar.copy(b1[:, g0:ge], pb[:, :gw])
                combine(h1, a1, b1, cs1, j0c, j1c, u_on_act=True)
                nc.sync.dma_start(
                    cin1[ch][:], h1[:, j0c:j1c].rearrange("p j b -> p (j b)"))
                nc.gpsimd.collective_compute(
                    "AllGather", Alu.bypass, replica_groups=shard_groups,
                    ins=[cin1[ch][:]],
                    outs=[g1[ch * ROWS1 : (ch + 1) * ROWS1, :]],
                )

            # prep b2 sub-gathers (desc-gen runs early; fired after AG)
            for q, (s0, s1, j0, j1) in enumerate(GSUB):
                nc.gpsimd.dma_gather(
                    b2[:, j0:j1], g1[:], i2sb[:, s0 // 16 : s1 // 16],
                    s1 - s0, s1 - s0, BC,
                    prepare_only=True, sem=sems[q], queue_num=q,
                    single_packet=False,
                )

            def pe_a_gather(nj, nb, oh_d, src, ns_src, nj_src, ns_out, a_sb):
                for gi, (g0, ge) in enumerate(jgroups(0, nj)):
                    gw = ge - g0
                    ohg = strm.tile([P, GJ * NB3 * P], fp8, tag="ohg")
                    nc.sync.dma_start(
                        ohg[:, : gw * nb * P],
                        oh_d[:, g0 * nb * P : ge * nb * P],
                    )
                    pa = psum.tile(
                        [P, GJ, BC], f32, tag="pa" if gi % 2 == 0 else "pb")
                    for jl in range(gw):
                        jj = g0 + jl
                        k0 = _k0(jj, ns_src, nj_src, ns_out, nb)
                        for v in range(nb):
                            nc.tensor.matmul(
                                pa[:, jl],
                                ohg[:, (jl * nb + v) * P : (jl * nb + v + 1) * P],
                                src[:, k0 + v],
                                start=(v == 0), stop=(v == nb - 1),
                            )
                    nc.scalar.copy(a_sb[:, g0:ge], pa[:, :gw])

            # ================= LAYER 2 =================
            cs2 = coeffs(w2p, NJ2, "b")
            h2 = big.tile([P, NJ2, BC], fp8, tag="h2")
            a2 = big.tile([P, NJ2, BC], fp8, tag="aL")
            pe_a_gather(NJ2, NB2, oh2, h1, NS1, NJ1, NS2, a2)

            for q in range(4):
                nc.gpsimd.trigger_dma(count=None, queue_num=q)

            w2s = [combine_w(a2, cs2, ch * JCH2, (ch + 1) * JCH2, ch)
                   for ch in range(NCH)]
            for ch in range(NCH):
                j0c, j1c = ch * JCH2, (ch + 1) * JCH2
                combine_rest(h2, w2s[ch], a2, b2, cs2, j0c, j1c)
                nc.sync.dma_start(
                    cin2[ch][:], h2[:, j0c:j1c].rearrange("p j b -> p (j b)"))
                nc.gpsimd.collective_compute(
                    "AllGather", Alu.bypass, replica_groups=shard_groups,
                    ins=[cin2[ch][:]],
                    outs=[g2[ch * ROWS2 : (ch + 1) * ROWS2, :]],
                )

            for q, (s0, s1, j0, j1) in enumerate(GSUB):
                nc.gpsimd.dma_gather(
                    b3[:, j0:j1], g2[:], i3sb[:, s0 // 16 : s1 // 16],
                    s1 - s0, s1 - s0, BC,
                    prepare_only=True, sem=sems[4 + q], queue_num=q,
                    single_packet=False,
                )

            # ================= LAYER 3 =================
            cs3 = coeffs(w3p, NJ3, "a")
            h3 = big.tile([P, NJ3, BC], fp8, tag="hA")
            a3 = big.tile([P, NJ3, BC], fp8, tag="aL")
            pe_a_gather(NJ3, NB3, oh3, h2, NS2, NJ2, NS3, a3)

            for q in range(4):
                nc.gpsimd.trigger_dma(count=None, queue_num=q)

            pg = psum.tile([NGROUP, BC], f32, tag="pg")
            w3s = [combine_w(a3, cs3, ch * JCH3, (ch + 1) * JCH3, ch)
                   for ch in range(NCH)]
            for ch in range(NCH):
                j0c, j1c = ch * JCH3, (ch + 1) * JCH3
                combine_rest(h3, w3s[ch], a3, b3, cs3, j0c, j1c)
                for j in range(j0c, j1c):
                    nc.tensor.matmul(
                        pg[:], gm[:, j], h3[:, j],
                        start=(j == 0), stop=(j == NJ3 - 1),
                    )

            # ---- cross-shard reduction + output ----
            psc = big.tile([NGROUP, BC], f32, tag="psc")
            nc.scalar.copy(psc[:], pg[:])
            nc.sync.dma_start(pin[:], psc[:])
            nc.gpsimd.collective_compute(
                "AllGather", Alu.bypass, replica_groups=shard_groups,
                ins=[pin[:]], outs=[pall[:]],
            )
            pall_sb = big.tile([1, SH, NGROUP * BC], f32, tag="pall_sb")
            nc.sync.dma_start(pall_sb[:], pall[:].rearrange("s u -> (s u)"))
            osb = big.tile([1, NGROUP * BC], f32, tag="osb")
            nc.vector.tensor_add(osb[:], pall_sb[:, 0], pall_sb[:, 1])
            nc.vector.tensor_add(osb[:], osb[:], pall_sb[:, 2])
            nc.vector.tensor_add(osb[:], osb[:], pall_sb[:, 3])
            nc.scalar.mul(osb[:], osb[:], 1.0 / TAU)
            # consume the warm-up collective output so DCE keeps it
            wsb2 = small.tile([1, 16], f32, tag="wsb2")
            nc.sync.dma_start(wsb2[:], warm[0:1, :])
            nc.vector.tensor_add(osb[:, :16], osb[:, :16], wsb2[:])
            nc.sync.dma_start(out_d[:], osb[:])

    # ---- surgical sync rewiring for the prepare_only gathers ----
    # Tile places the g-buffer RAW dep (Collectives sem) on the prep itself
    # (blocking desc-gen until the AllGather lands) and leaves consumers
    # waiting on a DMASW lane sem that nothing increments (the SDMA completion
    # bumps the sem= baked into the descriptors instead).  Move the
    # Collectives waits prep -> trigger and repoint the orphaned DMASW waits
    # at the descriptor completion sems.
    import bass_rust

    insts = list(nc.all_instructions())
    preps = [
        i for i in insts
        if type(i).__name__ == "InstDMAGatherAnt" and getattr(i, "gen_mode", 0) == 1
    ]
    assert len(preps) == 8, len(preps)
    # triggers in program order, paired per queue with preps in order
    trig_by_q = {}
    for i in insts:
        if type(i).__name__ == "InstTriggerDma":
            trig_by_q.setdefault(i.queue_num, []).append(i)
    prep_by_q = {}
    for p in preps:
        prep_by_q.setdefault(p.queue_num, []).append(p)
    updated = set()
    for i in insts:
        if i.sync_info:
            for u_ in (i.sync_info.on_update or []):
                updated.add(u_.ant_name)
    lane_sem = {}
    for li, p in enumerate(preps):
        lane_sem[f"DMASW{li % 8}"] = p.sync_info.on_update[0]
    for q, plist in prep_by_q.items():
        tlist = trig_by_q[q]
        assert len(plist) == len(tlist), (q, len(plist), len(tlist))
        for p, t in zip(plist, tlist):
            keep, move = [], []
            for w in p.sync_info.on_wait or []:
                (move if w.ant_name.startswith("Collectives") else keep).append(w)
            p.sync_info.on_wait = keep
            t.sync_info.on_wait = list(t.sync_info.on_wait or []) + move
    n_rewired = 0
    for i in insts:
        si = i.sync_info
        if not si or not si.on_wait:
            continue
        nw, changed = [], False
        for w in si.on_wait:
            base = w.ant_name.rsplit("_", 1)[0]
            if (
                w.ant_name.startswith("DMASW")
                and w.ant_name not in updated
                and base in lane_sem
            ):
                u_ = lane_sem[base]
                nw.append(bass_rust.SyncWait(
                    sync_type="semaphore", id=u_.id, ant_name=u_.ant_name,
                    wait_mode=w.wait_mode, wait_value=w.wait_value, wait_reg=None,
                ))
                changed = True
                n_rewired += 1
            else:
                nw.append(w)
        if changed:
            si.on_wait = nw
    assert n_rewired >= 8, f"expected orphan DMASW waits, rewired {n_rewired}"

    nc.compile()
    return nc


# ===================== host packing =====================

def _wrap_idx(ii):
    w = ii.astype(np.int16).reshape(-1, 16).T
    return np.ascontiguousarray(np.tile(w, (8, 1)))


def _pack_w(w_eff, nj):
    # slot t = j*128 + p  ->  packed[p, j*16+g]
    return np.ascontiguousarray(
        w_eff.reshape(nj, P, 16).transpose(1, 0, 2).reshape(P, nj * 16)
    )


PAD_ROW = np.full(16, -20.0, dtype=np.float32)
PAD_ROW[0] = 20.0  # softmax -> ~one-hot FALSE gate -> h = 0


def _assign_slots(la, ns_out):
    """Sort outputs by local a-slot, spread pads uniformly."""
    n = len(la)
    assert n <= ns_out, f"shard overflow: {n} > {ns_out}"
    order = np.argsort(la, kind="stable")
    slots = np.empty(n, dtype=np.int64)
    slots[order] = (np.arange(n, dtype=np.int64) * ns_out) // n
    return slots


def _onehot_pack(la_by_slot, ns_src, nj_src, ns_out, nb):
    """[P, NT*nb*P] uint8 one-hot lhsT data (0x00 / 0x38==1.0 e4m3)."""
    nt = ns_out // P
    oh = np.zeros((P, nt * nb * P), dtype=np.uint8)
    for jj in range(nt):
        k0 = _k0(jj, ns_src, nj_src, ns_out, nb)
        for m in range(P):
            la = la_by_slot[jj * P + m]
            if la < 0:
                continue
            r = la - k0 * P
            v, rr = divmod(r, P)
            assert 0 <= v < nb, f"window violation jj={jj} m={m} la={la} k0={k0}"
            oh[rr, (jj * nb + v) * P + m] = 0x38
    return oh.view(FP8)


def _host_pack(inputs):
    x = np.asarray(inputs["x"], dtype=np.float32)
    w1 = np.asarray(inputs["w1"], dtype=np.float32)
    w2 = np.asarray(inputs["w2"], dtype=np.float32)
    w3 = np.asarray(inputs["w3"], dtype=np.float32)
    i1a = np.asarray(inputs["idx1a"]).astype(np.int64)
    i1b = np.asarray(inputs["idx1b"]).astype(np.int64)
    i2a = np.asarray(inputs["idx2a"]).astype(np.int64)
    i2b = np.asarray(inputs["idx2b"]).astype(np.int64)
    i3a = np.asarray(inputs["idx3a"]).astype(np.int64)
    i3b = np.asarray(inputs["idx3b"]).astype(np.int64)

    per_shard = [dict() for _ in range(SH)]

    # ---------- layer 1: contiguous assignment ----------
    for s in range(SH):
        sel = np.arange(s * R1, (s + 1) * R1)
        w_eff = np.concatenate(
            [w1[sel], np.tile(PAD_ROW, (NS1 - R1, 1))], axis=0
        )
        per_shard[s]["w1p"] = _pack_w(w_eff, NJ1)
        for nm, idx in (("oh1a", i1a), ("oh1b", i1b)):
            oh = np.zeros((P, NJ1 * 2 * P), dtype=np.uint8)
            t = np.arange(R1)
            jj, m = t // P, t % P
            src = idx[s * R1 + t]
            k, rr = src // P, src % P
            oh[rr, (jj * 2 + k) * P + m] = 0x38
            per_shard[s][nm] = oh.view(FP8)

    # ---------- layer 2 ----------
    s2 = i2a // R1
    la2 = i2a - s2 * R1
    slot2 = np.zeros(L1N, dtype=np.int64)
    for s in range(SH):
        sel = np.where(s2 == s)[0]
        sl = _assign_slots(la2[sel], NS2)
        slot2[sel] = s * NS2 + sl
        la_by_slot = np.full(NS2, -1, dtype=np.int64)
        la_by_slot[sl] = la2[sel]
        per_shard[s]["oh2"] = _onehot_pack(la_by_slot, NS1, NJ1, NS2, NB2)
        w_eff = np.tile(PAD_ROW, (NS2, 1))
        w_eff[sl] = w2[sel]
        per_shard[s]["w2p"] = _pack_w(w_eff, NJ2)
        ib = i2b[sel]
        sb, tb = ib // R1, ib % R1
        jb, pb = tb // P, tb % P
        ch = jb // JCH1
        row = ((ch * SH + sb) * P + pb) * JCH1 + (jb - ch * JCH1)
        idx_eff = np.zeros(NS2, dtype=np.int64)
        idx_eff[sl] = row
        per_shard[s]["i2"] = _wrap_idx(idx_eff)

    # ---------- layer 3 ----------
    g3 = slot2[i3a]
    s3 = g3 // NS2
    la3 = g3 - s3 * NS2
    grp = np.arange(L3N) // SPG
    for s in range(SH):
        sel = np.where(s3 == s)[0]
        sl = _assign_slots(la3[sel], NS3)
        la_by_slot = np.full(NS3, -1, dtype=np.int64)
        la_by_slot[sl] = la3[sel]
        per_shard[s]["oh3"] = _onehot_pack(la_by_slot, NS2, NJ2, NS3, NB3)
        w_eff = np.tile(PAD_ROW, (NS3, 1))
        w_eff[sl] = w3[sel]
        per_shard[s]["w3p"] = _pack_w(w_eff, NJ3)
        gb = slot2[i3b[sel]]
        sb, tb = gb // NS2, gb % NS2
        jb, pb = tb // P, tb % P
        ch = jb // JCH2
        row = ((ch * SH + sb) * P + pb) * JCH2 + (jb - ch * JCH2)
        idx_eff = np.zeros(NS3, dtype=np.int64)
        idx_eff[sl] = row
        per_shard[s]["i3"] = _wrap_idx(idx_eff)
        gmask = np.zeros((NS3, NGROUP), dtype=np.uint8)
        gmask[sl, grp[sel]] = 0x38
        per_shard[s]["gmask"] = np.ascontiguousarray(
            gmask.reshape(NJ3, P, NGROUP).transpose(1, 0, 2).reshape(P, NJ3 * NGROUP)
        ).view(FP8)

    in_maps = []
    for c in range(N_CORES):
        G, s = c // SH, c % SH
        m_ = dict(per_shard[s])
        m_["xT"] = np.ascontiguousarray(x[G * BC : (G + 1) * BC].T)
        in_maps.append(m_)
    return in_maps


LAST_RESULTS = None


def kernel(**inputs):
    global LAST_RESULTS
    from concourse.bass_utils import run_bass_kernel_spmd

    if "nc" not in _CACHE:
        _CACHE["nc"] = _build_nc()
    nc = _CACHE["nc"]

    in_maps = _host_pack(inputs)
    trace = bool(int(os.environ.get("KERNEL_TRACE", "0")))
    res = run_bass_kernel_spmd(
        nc, in_maps, core_ids=list(range(N_CORES)), trace=trace
    )
    LAST_RESULTS = res

    out = np.empty((B, NGROUP), dtype=np.float32)
    for g_ in range(BG):
        rc = res.results[g_ * SH]["out"].reshape(NGROUP, BC)
        out[g_ * BC : (g_ + 1) * BC, :] = rc.T
    return out
